# revision 9
# baseline (speedup 1.0000x reference)
"""Trainium2 Bass kernel for nn_MultiHeadedAttention_64665027608991.

Sparse (per-frame-masked) multi-head attention over B=512 samples, L=176
(8 frames x 22 joints), 8 heads x 64 dims, fp32 I/O.

Strategy: pure data parallel over batch (64 samples per NeuronCore x 8).
Per sample, fully unrolled:
  - x^T (host-pre-transposed) -> q^T/k^T via fp32r matmuls (tf32-class),
    biases folded into the PSUM->SBUF copy on ScalarE.
  - v natural layout with bias via K=1 ones matmul, ReLU on ScalarE into a
    ones-augmented bf16 tile (65 cols per head; col 64 = 1.0 for row sums).
  - scores S^T[k,q] per head via fp32r matmuls; the temporal mask is added
    in PSUM by a constant-matrix matmul (lhsT=I88, rhs=-100 on masked pairs);
    exp on ScalarE (no max subtraction: |scores| <= ~3) -> bf16 P^T.
  - O^T = [v|1]^T @ P^T (bf16): row 64 gives softmax denominators; recip on
    VectorE, broadcast via K=1 matmul, normalize on VectorE.
  - final projection from O^T slices (fp32r) + bias, DMA out.
"""

import sys

sys.path.insert(0, "/opt/trn_rl_repo")

import json

import numpy as np

import concourse.bass as bass
import concourse.tile as tile
from concourse import mybir
from concourse.bass_utils import run_bass_kernel_spmd

DT = mybir.dt

N_CORES = 8
B = 512
BS = B // N_CORES  # 64 samples per core
L = 176
FRAME = 22
NFRAME = 8
IN_DIM = 128
D_MODEL = 512
H_NUM = 8
H_DIM = 64
OUT_DIM = 512
SCALE = 1.0 / np.sqrt(np.float32(H_DIM))
NEG = -100.0  # exp(s + NEG) < 1e-40 for |s| < 10 -> masked weights vanish


# ---------------------------------------------------------------------------
# Workaround: the walrus build in this container rejects instructions with
# more than one sync-wait. Split extras onto single-wait EventSemaphore
# carriers on the same engine.
def _split_multiwaits(bir_json_bytes: bytes) -> bytes:
    j = json.loads(bir_json_bytes)
    n = [0]

    def fix_block(b):
        insts = b.get("instructions")
        if insts:
            out = []
            for inst in insts:
                si = inst.get("sync_info")
                waits = (si or {}).get("on_wait") or []
                if len(waits) > 1:
                    for w in waits[:-1]:
                        n[0] += 1
                        out.append({
                            "name": f"waitfix_{n[0]}",
                            "opcode": "EventSemaphore",
                            "engine": inst.get("engine"),
                            "ins": [],
                            "outs": [],
                            "sync_info": {"on_update": [], "on_wait": [w]},
                        })
                    si["on_wait"] = [waits[-1]]
                out.append(inst)
            b["instructions"] = out
        for sub in b.get("blocks", []) or []:
            fix_block(sub)

    for fn in j["functions"]:
        for blk in fn["blocks"]:
            fix_block(blk)
    return json.dumps(j).encode()


def _install_waitfix(nc):
    orig = nc.to_json_bytes
    nc.to_json_bytes = lambda: _split_multiwaits(orig())


def _build_nc():
    nc = bass.Bass(trn_type="TRN2", debug=False)
    _install_waitfix(nc)
    f32, f32r, bf16 = DT.float32, DT.float32r, DT.bfloat16

    xT_d = nc.dram_tensor("xT", [BS, IN_DIM, L], f32r, kind="ExternalInput")
    wq_d = nc.dram_tensor("wq", [IN_DIM, D_MODEL], f32r, kind="ExternalInput")
    wk_d = nc.dram_tensor("wk", [IN_DIM, D_MODEL], f32r, kind="ExternalInput")
    wv_d = nc.dram_tensor("wv", [IN_DIM, D_MODEL], f32r, kind="ExternalInput")
    wf_d = nc.dram_tensor("wf", [4, IN_DIM, OUT_DIM], f32r, kind="ExternalInput")
    bq_d = nc.dram_tensor("bq", [IN_DIM, 4], f32, kind="ExternalInput")
    bk_d = nc.dram_tensor("bk", [IN_DIM, 4], f32, kind="ExternalInput")
    bv_d = nc.dram_tensor("bv", [1, D_MODEL], f32r, kind="ExternalInput")
    bf_d = nc.dram_tensor("bf", [1, OUT_DIM], f32r, kind="ExternalInput")
    madd_d = nc.dram_tensor("madd", [2, 88, L], bf16, kind="ExternalInput")
    i88_d = nc.dram_tensor("i88", [88, 88], bf16, kind="ExternalInput")
    ones_d = nc.dram_tensor("ones", [1, IN_DIM], f32r, kind="ExternalInput")
    y_d = nc.dram_tensor("y", [BS, L, OUT_DIM], f32, kind="ExternalOutput")

    Copy = mybir.ActivationFunctionType.Copy
    Ident = mybir.ActivationFunctionType.Identity
    Exp = mybir.ActivationFunctionType.Exp
    Relu = mybir.ActivationFunctionType.Relu

    with tile.TileContext(nc) as tc:
        with (
            tc.tile_pool(name="consts", bufs=1) as cp,
            tc.tile_pool(name="xp", bufs=2) as xp,
            tc.tile_pool(name="qk", bufs=2) as qkp,
            tc.tile_pool(name="vp", bufs=2) as vp,
            tc.tile_pool(name="ptp", bufs=3) as ptp,
            tc.tile_pool(name="osb", bufs=2) as osbp,
            tc.tile_pool(name="recp", bufs=2) as recp,
            tc.tile_pool(name="yp", bufs=2) as yp,
            tc.tile_pool(name="ps_qk", bufs=2, space="PSUM") as pp_qk,
            tc.tile_pool(name="ps_v", bufs=1, space="PSUM") as pp_v,
            tc.tile_pool(name="ps_s", bufs=2, space="PSUM") as pp_s,
            tc.tile_pool(name="ps_o", bufs=1, space="PSUM") as pp_o,
            tc.tile_pool(name="ps_b", bufs=1, space="PSUM") as pp_b,
            tc.tile_pool(name="ps_y", bufs=1, space="PSUM") as pp_y,
        ):
            wq = cp.tile([IN_DIM, D_MODEL], f32r)
            nc.sync.dma_start(wq[:], wq_d.ap()[:])
            wk = cp.tile([IN_DIM, D_MODEL], f32r)
            nc.sync.dma_start(wk[:], wk_d.ap()[:])
            wv = cp.tile([IN_DIM, D_MODEL], f32r)
            nc.sync.dma_start(wv[:], wv_d.ap()[:])
            wf = cp.tile([IN_DIM, 4 * OUT_DIM], f32r)
            for c in range(4):
                nc.sync.dma_start(wf[:, 512 * c:512 * (c + 1)], wf_d.ap()[c])
            bq = cp.tile([IN_DIM, 4], f32)
            nc.sync.dma_start(bq[:], bq_d.ap()[:])
            bk = cp.tile([IN_DIM, 4], f32)
            nc.sync.dma_start(bk[:], bk_d.ap()[:])
            bv = cp.tile([1, D_MODEL], f32r)
            nc.sync.dma_start(bv[:], bv_d.ap()[:])
            bf_t = cp.tile([1, OUT_DIM], f32r)
            nc.sync.dma_start(bf_t[:], bf_d.ap()[:])
            madd = cp.tile([88, 2 * L], bf16)
            for kc in range(2):
                nc.sync.dma_start(madd[:, L * kc:L * (kc + 1)], madd_d.ap()[kc])
            i88 = cp.tile([88, 88], bf16)
            nc.sync.dma_start(i88[:], i88_d.ap()[:])
            ones = cp.tile([1, IN_DIM], f32r)
            nc.sync.dma_start(ones[:], ones_d.ap()[:])

            for s in range(BS):
                xt = xp.tile([IN_DIM, L], f32r)
                nc.sync.dma_start(xt[:], xT_d.ap()[s])

                # q^T / k^T projections: psum [128, 176] per 128-chunk of
                # d_model; bias added during PSUM->SBUF copy on ScalarE.
                qt = qkp.tile([IN_DIM, 4 * L], f32r, name="qt")
                kt = qkp.tile([IN_DIM, 4 * L], f32r, name="kt")
                for w_t, b_t, dst in ((wq, bq, qt), (wk, bk, kt)):
                    for c in range(4):
                        pq = pp_qk.tile([IN_DIM, L], f32, name="pq")
                        nc.tensor.matmul(
                            pq[:], w_t[:, 128 * c:128 * (c + 1)], xt[:],
                            start=True, stop=True,
                        )
                        nc.scalar.activation(
                            dst[:, L * c:L * (c + 1)], pq[:], Ident,
                            bias=b_t[:, c:c + 1],
                        )

                # v: natural layout, keys on partitions, ones-augmented bf16
                va = []
                for rc in range(2):
                    pv = pp_v.tile([88, D_MODEL], f32, name="pv")
                    nc.tensor.matmul(
                        pv[:], xt[:, 88 * rc:88 * (rc + 1)], wv[:],
                        start=True, stop=False,
                    )
                    nc.tensor.matmul(
                        pv[:], ones[:, 0:88], bv[:], start=False, stop=True,
                    )
                    vt = vp.tile([88, 8 * 65], bf16, name=f"va{rc}")
                    vv = vt[:].rearrange("p (h w) -> p h w", w=65)
                    pvv = pv[:].rearrange("p (h w) -> p h w", w=64)
                    nc.scalar.activation(vv[:, :, 0:64], pvv[:], Relu)
                    nc.gpsimd.memset(vv[:, :, 64:65], 1.0)
                    va.append(vt)

                osb = osbp.tile([IN_DIM, 4 * L], f32r, name="osb")
                for h in range(H_NUM):
                    hp, hr = h >> 1, 64 * (h & 1)
                    sp = pp_s.tile([88, 2 * L], f32, name="sp")
                    for kc in range(2):
                        nc.tensor.matmul(
                            sp[:, L * kc:L * (kc + 1)],
                            kt[hr:hr + 64, L * hp + 88 * kc:L * hp + 88 * (kc + 1)],
                            qt[hr:hr + 64, L * hp:L * (hp + 1)],
                            start=True, stop=False,
                        )
                        nc.tensor.matmul(
                            sp[:, L * kc:L * (kc + 1)],
                            i88[:], madd[:, L * kc:L * (kc + 1)],
                            start=False, stop=True,
                        )
                    pt = ptp.tile([88, 2 * L], bf16, name="pt")
                    nc.scalar.activation(pt[:], sp[:], Exp)

                    po = pp_o.tile([65, L], f32, name="po")
                    for kc in range(2):
                        nc.tensor.matmul(
                            po[:], va[kc][:, 65 * h:65 * h + 65],
                            pt[:, L * kc:L * (kc + 1)],
                            start=(kc == 0), stop=(kc == 1),
                        )
                    rec = recp.tile([1, L], f32r, name="rec")
                    with nc.allow_low_precision(reason="f32r recip feeds f32r bcast matmul"):
                        nc.vector.reciprocal(rec[:], po[64:65, :])
                    pb = pp_b.tile([64, L], f32, name="pb")
                    nc.tensor.matmul(pb[:], ones[:, 0:64], rec[:],
                                     start=True, stop=True)
                    dst = osb[hr:hr + 64, L * hp:L * (hp + 1)]
                    nc.scalar.activation(dst, po[0:64, :], Copy)
                    nc.vector.tensor_mul(dst, dst, pb[:])

                for rc in range(2):
                    py = pp_y.tile([88, OUT_DIM], f32, name="py")
                    for c in range(4):
                        nc.tensor.matmul(
                            py[:],
                            osb[:, L * c + 88 * rc:L * c + 88 * (rc + 1)],
                            wf[:, 512 * c:512 * (c + 1)],
                            start=(c == 0), stop=False,
                        )
                    nc.tensor.matmul(py[:], ones[:, 0:88], bf_t[:],
                                     start=False, stop=True)
                    ysb = yp.tile([88, OUT_DIM], f32, name="ysb")
                    nc.vector.tensor_copy(ysb[:], py[:])
                    nc.sync.dma_start(
                        y_d.ap()[s, 88 * rc:88 * (rc + 1), :], ysb[:],
                    )
    return nc


def _make_consts():
    frame = np.arange(L) // FRAME
    same_frame = frame[:, None] == frame[None, :]
    madd = np.where(same_frame & ~np.eye(L, dtype=bool), np.float32(NEG),
                    np.float32(0.0))
    import ml_dtypes
    return {
        "madd": np.stack([madd[0:88], madd[88:176]]).astype(ml_dtypes.bfloat16),
        "i88": np.eye(88, dtype=np.float32).astype(ml_dtypes.bfloat16),
        "ones": np.ones((1, IN_DIM), dtype=np.float32),
    }


_NC_CACHE = None


def kernel(x, Wq, bq, Wk, bk, Wv, bv, Wf, bf):
    global _NC_CACHE
    x = np.asarray(x, dtype=np.float32)
    if _NC_CACHE is None:
        _NC_CACHE = _build_nc()
    nc = _NC_CACHE

    consts = _make_consts()
    xT = np.ascontiguousarray(x.transpose(0, 2, 1))  # [B, 128, 176]
    base = {
        "wq": np.asarray(Wq, np.float32) * SCALE,  # fold 1/sqrt(H_DIM) into q
        "wk": np.asarray(Wk, np.float32),
        "wv": np.asarray(Wv, np.float32),
        "wf": np.ascontiguousarray(
            np.asarray(Wf, np.float32).reshape(4, IN_DIM, OUT_DIM)),
        "bq": np.ascontiguousarray(
            (np.asarray(bq, np.float32) * SCALE).reshape(4, IN_DIM).T),
        "bk": np.ascontiguousarray(np.asarray(bk, np.float32).reshape(4, IN_DIM).T),
        "bv": np.asarray(bv, np.float32).reshape(1, D_MODEL),
        "bf": np.asarray(bf, np.float32).reshape(1, OUT_DIM),
        **consts,
    }
    in_maps = [
        {**base, "xT": np.ascontiguousarray(xT[BS * c:BS * (c + 1)])}
        for c in range(N_CORES)
    ]
    res = run_bass_kernel_spmd(nc, in_maps, core_ids=list(range(N_CORES)))
    return np.concatenate([r["y"] for r in res.results], axis=0)


# revision 20
# speedup vs baseline: 1.0608x; 1.0608x over previous
"""Trainium2 Bass kernel for nn_MultiHeadedAttention_64665027608991.

Sparse (per-frame-masked) multi-head attention over B=512 samples, L=176
(8 frames x 22 joints), 8 heads x 64 dims, fp32 I/O.

Strategy: pure data parallel over batch (64 samples per NeuronCore x 8).
Per sample, fully unrolled:
  - x^T (host-pre-transposed) -> q^T/k^T via fp32r matmuls (tf32-class),
    biases folded into the PSUM->SBUF copy on ScalarE.
  - v natural layout with bias via K=1 ones matmul, ReLU on ScalarE into a
    ones-augmented bf16 tile (65 cols per head; col 64 = 1.0 for row sums).
  - scores S^T[k,q] per head via fp32r matmuls; the temporal mask is added
    in PSUM by a constant-matrix matmul (lhsT=I88, rhs=-100 on masked pairs);
    exp on ScalarE (no max subtraction: |scores| <= ~3) -> bf16 P^T.
  - O^T = [v|1]^T @ P^T (bf16): row 64 gives softmax denominators; recip on
    VectorE, broadcast via K=1 matmul, normalize on VectorE.
  - final projection from O^T slices (fp32r) + bias, DMA out.
"""

import sys

sys.path.insert(0, "/opt/trn_rl_repo")

import json

import numpy as np

import concourse.bass as bass
import concourse.tile as tile
from concourse import mybir
from concourse.bass_utils import run_bass_kernel_spmd

DT = mybir.dt

N_CORES = 8
B = 512
BS = B // N_CORES  # 64 samples per core
L = 176
FRAME = 22
NFRAME = 8
IN_DIM = 128
D_MODEL = 512
H_NUM = 8
H_DIM = 64
OUT_DIM = 512
SCALE = 1.0 / np.sqrt(np.float32(H_DIM))
NEG = -100.0  # exp(s + NEG) < 1e-40 for |s| < 10 -> masked weights vanish


# ---------------------------------------------------------------------------
# Workaround: the walrus build in this container rejects instructions with
# more than one sync-wait. Split extras onto single-wait EventSemaphore
# carriers on the same engine.
def _split_multiwaits(bir_json_bytes: bytes) -> bytes:
    j = json.loads(bir_json_bytes)
    n = [0]

    def fix_block(b):
        insts = b.get("instructions")
        if insts:
            out = []
            for inst in insts:
                si = inst.get("sync_info")
                waits = (si or {}).get("on_wait") or []
                if len(waits) > 1:
                    for w in waits[:-1]:
                        n[0] += 1
                        out.append({
                            "name": f"waitfix_{n[0]}",
                            "opcode": "EventSemaphore",
                            "engine": inst.get("engine"),
                            "ins": [],
                            "outs": [],
                            "sync_info": {"on_update": [], "on_wait": [w]},
                        })
                    si["on_wait"] = [waits[-1]]
                out.append(inst)
            b["instructions"] = out
        for sub in b.get("blocks", []) or []:
            fix_block(sub)

    for fn in j["functions"]:
        for blk in fn["blocks"]:
            fix_block(blk)
    return json.dumps(j).encode()


def _install_waitfix(nc):
    orig = nc.to_json_bytes
    nc.to_json_bytes = lambda: _split_multiwaits(orig())


def _build_nc(repeat=1):
    nc = bass.Bass(trn_type="TRN2", debug=False)
    _install_waitfix(nc)
    f32, f32r, bf16 = DT.float32, DT.float32r, DT.bfloat16

    xT_d = nc.dram_tensor("xT", [BS, IN_DIM, L], f32r, kind="ExternalInput")
    wq_d = nc.dram_tensor("wq", [IN_DIM, D_MODEL], f32r, kind="ExternalInput")
    wk_d = nc.dram_tensor("wk", [IN_DIM, D_MODEL], f32r, kind="ExternalInput")
    wv_d = nc.dram_tensor("wv", [IN_DIM, D_MODEL], f32r, kind="ExternalInput")
    wf_d = nc.dram_tensor("wf", [4, IN_DIM, OUT_DIM], f32r, kind="ExternalInput")
    bq_d = nc.dram_tensor("bq", [IN_DIM, 4], f32, kind="ExternalInput")
    bk_d = nc.dram_tensor("bk", [IN_DIM, 4], f32, kind="ExternalInput")
    bv_d = nc.dram_tensor("bv", [1, D_MODEL], f32r, kind="ExternalInput")
    bf_d = nc.dram_tensor("bf", [1, OUT_DIM], f32r, kind="ExternalInput")
    mask_d = nc.dram_tensor("mask01", [2, 88, L], bf16, kind="ExternalInput")
    ones_d = nc.dram_tensor("ones", [1, IN_DIM], f32r, kind="ExternalInput")
    y_d = nc.dram_tensor("y", [BS, L, OUT_DIM], f32, kind="ExternalOutput")

    Copy = mybir.ActivationFunctionType.Copy
    Ident = mybir.ActivationFunctionType.Identity
    Exp = mybir.ActivationFunctionType.Exp
    Relu = mybir.ActivationFunctionType.Relu

    with tile.TileContext(nc) as tc:
        with (
            tc.tile_pool(name="consts", bufs=1) as cp,
            tc.tile_pool(name="xp", bufs=2) as xp,
            tc.tile_pool(name="qk", bufs=2) as qkp,
            tc.tile_pool(name="vp", bufs=2) as vp,
            tc.tile_pool(name="ptp", bufs=3) as ptp,
            tc.tile_pool(name="osb", bufs=2) as osbp,
            tc.tile_pool(name="recp", bufs=2) as recp,
            tc.tile_pool(name="yp", bufs=2) as yp,
            tc.tile_pool(name="ps_qk", bufs=2, space="PSUM") as pp_qk,
            tc.tile_pool(name="ps_v", bufs=1, space="PSUM") as pp_v,
            tc.tile_pool(name="ps_s", bufs=2, space="PSUM") as pp_s,
            tc.tile_pool(name="ps_o", bufs=1, space="PSUM") as pp_o,
            tc.tile_pool(name="ps_b", bufs=1, space="PSUM") as pp_b,
            tc.tile_pool(name="ps_y", bufs=1, space="PSUM") as pp_y,
        ):
            wq = cp.tile([IN_DIM, D_MODEL], f32r)
            nc.sync.dma_start(wq[:], wq_d.ap()[:])
            wk = cp.tile([IN_DIM, D_MODEL], f32r)
            nc.sync.dma_start(wk[:], wk_d.ap()[:])
            wv = cp.tile([IN_DIM, D_MODEL], f32r)
            nc.sync.dma_start(wv[:], wv_d.ap()[:])
            wf = cp.tile([IN_DIM, 4 * OUT_DIM], f32r)
            for c in range(4):
                nc.sync.dma_start(wf[:, 512 * c:512 * (c + 1)], wf_d.ap()[c])
            bq = cp.tile([IN_DIM, 4], f32)
            nc.sync.dma_start(bq[:], bq_d.ap()[:])
            bk = cp.tile([IN_DIM, 4], f32)
            nc.sync.dma_start(bk[:], bk_d.ap()[:])
            bv = cp.tile([1, D_MODEL], f32r)
            nc.sync.dma_start(bv[:], bv_d.ap()[:])
            bf_t = cp.tile([1, OUT_DIM], f32r)
            nc.sync.dma_start(bf_t[:], bf_d.ap()[:])
            mask01 = cp.tile([88, 2 * L], bf16)
            for kc in range(2):
                nc.sync.dma_start(mask01[:, L * kc:L * (kc + 1)], mask_d.ap()[kc])
            ones = cp.tile([1, IN_DIM], f32r)
            nc.sync.dma_start(ones[:], ones_d.ap()[:])

            for s in range(BS * repeat):
                s = s % BS
                xt = xp.tile([IN_DIM, L], f32r)
                nc.sync.dma_start(xt[:], xT_d.ap()[s])

                # q^T / k^T projections: psum [128, 176] per 128-chunk of
                # d_model; bias added during PSUM->SBUF copy on ScalarE.
                qt = qkp.tile([IN_DIM, 4 * L], f32r, name="qt")
                kt = qkp.tile([IN_DIM, 4 * L], f32r, name="kt")
                for w_t, b_t, dst in ((wq, bq, qt), (wk, bk, kt)):
                    for c in range(4):
                        pq = pp_qk.tile([IN_DIM, L], f32, name="pq")
                        nc.tensor.matmul(
                            pq[:], w_t[:, 128 * c:128 * (c + 1)], xt[:],
                            start=True, stop=True,
                        )
                        nc.scalar.activation(
                            dst[:, L * c:L * (c + 1)], pq[:], Ident,
                            bias=b_t[:, c:c + 1],
                        )

                # v: natural layout, keys on partitions, ones-augmented bf16
                va = []
                for rc in range(2):
                    pv = pp_v.tile([88, D_MODEL], f32, name="pv")
                    nc.tensor.matmul(
                        pv[:], xt[:, 88 * rc:88 * (rc + 1)], wv[:],
                        start=True, stop=False,
                    )
                    nc.tensor.matmul(
                        pv[:], ones[:, 0:88], bv[:], start=False, stop=True,
                    )
                    vt = vp.tile([88, 8 * 65], bf16, name=f"va{rc}")
                    vv = vt[:].rearrange("p (h w) -> p h w", w=65)
                    pvv = pv[:].rearrange("p (h w) -> p h w", w=64)
                    nc.scalar.activation(vv[:, :, 0:64], pvv[:], Relu)
                    nc.gpsimd.memset(vv[:, :, 64:65], 1.0)
                    va.append(vt)

                osb = osbp.tile([IN_DIM, 4 * L], f32r, name="osb")
                for h in range(H_NUM):
                    hp, hr = h >> 1, 64 * (h & 1)
                    sp = pp_s.tile([88, 2 * L], f32, name="sp")
                    for kc in range(2):
                        nc.tensor.matmul(
                            sp[:, L * kc:L * (kc + 1)],
                            kt[hr:hr + 64, L * hp + 88 * kc:L * hp + 88 * (kc + 1)],
                            qt[hr:hr + 64, L * hp:L * (hp + 1)],
                            start=True, stop=True,
                        )
                    pt = ptp.tile([88, 2 * L], bf16, name="pt")
                    nc.scalar.activation(pt[:], sp[:], Exp)
                    nc.gpsimd.tensor_mul(pt[:], pt[:], mask01[:])

                    po = pp_o.tile([65, L], f32, name="po")
                    for kc in range(2):
                        nc.tensor.matmul(
                            po[:], va[kc][:, 65 * h:65 * h + 65],
                            pt[:, L * kc:L * (kc + 1)],
                            start=(kc == 0), stop=(kc == 1),
                        )
                    rec = recp.tile([1, L], f32r, name="rec")
                    with nc.allow_low_precision(reason="f32r recip feeds f32r bcast matmul"):
                        nc.vector.reciprocal(rec[:], po[64:65, :])
                    pb = pp_b.tile([64, L], f32, name="pb")
                    nc.tensor.matmul(pb[:], ones[:, 0:64], rec[:],
                                     start=True, stop=True)
                    dst = osb[hr:hr + 64, L * hp:L * (hp + 1)]
                    if h % 2 == 0:
                        nc.scalar.activation(dst, po[0:64, :], Copy)
                    else:
                        nc.vector.tensor_copy(dst, po[0:64, :])
                    nc.vector.tensor_mul(dst, dst, pb[:])

                for rc in range(2):
                    py = pp_y.tile([88, OUT_DIM], f32, name="py")
                    for c in range(4):
                        nc.tensor.matmul(
                            py[:],
                            osb[:, L * c + 88 * rc:L * c + 88 * (rc + 1)],
                            wf[:, 512 * c:512 * (c + 1)],
                            start=(c == 0), stop=False,
                        )
                    nc.tensor.matmul(py[:], ones[:, 0:88], bf_t[:],
                                     start=False, stop=True)
                    ysb = yp.tile([88, OUT_DIM], f32, name="ysb")
                    if rc == 0:
                        nc.vector.tensor_copy(ysb[:], py[:])
                    else:
                        nc.scalar.activation(ysb[:], py[:], Copy)
                    nc.sync.dma_start(
                        y_d.ap()[s, 88 * rc:88 * (rc + 1), :], ysb[:],
                    )
    return nc


def _make_consts():
    frame = np.arange(L) // FRAME
    same_frame = frame[:, None] == frame[None, :]
    mask01 = np.where(same_frame & ~np.eye(L, dtype=bool), np.float32(0.0),
                      np.float32(1.0))
    import ml_dtypes
    return {
        "mask01": np.stack([mask01[0:88], mask01[88:176]]).astype(
            ml_dtypes.bfloat16),
        "ones": np.ones((1, IN_DIM), dtype=np.float32),
    }


_NC_CACHE = None


def kernel(x, Wq, bq, Wk, bk, Wv, bv, Wf, bf):
    global _NC_CACHE
    x = np.asarray(x, dtype=np.float32)
    if _NC_CACHE is None:
        _NC_CACHE = _build_nc()
    nc = _NC_CACHE

    consts = _make_consts()
    xT = np.ascontiguousarray(x.transpose(0, 2, 1))  # [B, 128, 176]
    base = {
        "wq": np.asarray(Wq, np.float32) * SCALE,  # fold 1/sqrt(H_DIM) into q
        "wk": np.asarray(Wk, np.float32),
        "wv": np.asarray(Wv, np.float32),
        "wf": np.ascontiguousarray(
            np.asarray(Wf, np.float32).reshape(4, IN_DIM, OUT_DIM)),
        "bq": np.ascontiguousarray(
            (np.asarray(bq, np.float32) * SCALE).reshape(4, IN_DIM).T),
        "bk": np.ascontiguousarray(np.asarray(bk, np.float32).reshape(4, IN_DIM).T),
        "bv": np.asarray(bv, np.float32).reshape(1, D_MODEL),
        "bf": np.asarray(bf, np.float32).reshape(1, OUT_DIM),
        **consts,
    }
    in_maps = [
        {**base, "xT": np.ascontiguousarray(xT[BS * c:BS * (c + 1)])}
        for c in range(N_CORES)
    ]
    res = run_bass_kernel_spmd(nc, in_maps, core_ids=list(range(N_CORES)))
    return np.concatenate([r["y"] for r in res.results], axis=0)


# revision 30
# speedup vs baseline: 1.2263x; 1.1560x over previous
"""Trainium2 Bass kernel for nn_MultiHeadedAttention_64665027608991.

Sparse (per-frame-masked) multi-head attention over B=512 samples, L=176
(8 frames x 22 joints), 8 heads x 64 dims, fp32 I/O.

Strategy: pure data parallel over batch (64 samples per NeuronCore x 8).
Per sample, fully unrolled:
  - x^T (host-pre-transposed) -> q^T/k^T via fp32r matmuls (tf32-class),
    biases folded into the PSUM->SBUF copy on ScalarE.
  - v natural layout with bias via K=1 ones matmul, ReLU on ScalarE into a
    ones-augmented bf16 tile (65 cols per head; col 64 = 1.0 for row sums).
  - scores S^T[k,q] per head via fp32r matmuls; the temporal mask is added
    in PSUM by a constant-matrix matmul (lhsT=I88, rhs=-100 on masked pairs);
    exp on ScalarE (no max subtraction: |scores| <= ~3) -> bf16 P^T.
  - O^T = [v|1]^T @ P^T (bf16): row 64 gives softmax denominators; recip on
    VectorE, broadcast via K=1 matmul, normalize on VectorE.
  - final projection from O^T slices (fp32r) + bias, DMA out.
"""

import sys

sys.path.insert(0, "/opt/trn_rl_repo")

import json

import numpy as np

import concourse.bass as bass
import concourse.tile as tile
from concourse import mybir
from concourse.bass_utils import run_bass_kernel_spmd

DT = mybir.dt

N_CORES = 8
B = 512
BS = B // N_CORES  # 64 samples per core
L = 176
FRAME = 22
NFRAME = 8
IN_DIM = 128
D_MODEL = 512
H_NUM = 8
H_DIM = 64
OUT_DIM = 512
SCALE = 1.0 / np.sqrt(np.float32(H_DIM))
NEG = -100.0  # exp(s + NEG) < 1e-40 for |s| < 10 -> masked weights vanish


# ---------------------------------------------------------------------------
# Workaround: the walrus build in this container rejects instructions with
# more than one sync-wait. Split extras onto single-wait EventSemaphore
# carriers on the same engine.
def _split_multiwaits(bir_json_bytes: bytes) -> bytes:
    j = json.loads(bir_json_bytes)
    n = [0]

    def fix_block(b):
        insts = b.get("instructions")
        if insts:
            out = []
            for inst in insts:
                si = inst.get("sync_info")
                waits = (si or {}).get("on_wait") or []
                if len(waits) > 1:
                    for w in waits[:-1]:
                        n[0] += 1
                        out.append({
                            "name": f"waitfix_{n[0]}",
                            "opcode": "EventSemaphore",
                            "engine": inst.get("engine"),
                            "ins": [],
                            "outs": [],
                            "sync_info": {"on_update": [], "on_wait": [w]},
                        })
                    si["on_wait"] = [waits[-1]]
                out.append(inst)
            b["instructions"] = out
        for sub in b.get("blocks", []) or []:
            fix_block(sub)

    for fn in j["functions"]:
        for blk in fn["blocks"]:
            fix_block(blk)
    return json.dumps(j).encode()


def _install_waitfix(nc):
    orig = nc.to_json_bytes
    nc.to_json_bytes = lambda: _split_multiwaits(orig())


def _build_nc(repeat=1):
    nc = bass.Bass(trn_type="TRN2", debug=False)
    _install_waitfix(nc)
    f32, f32r, bf16 = DT.float32, DT.float32r, DT.bfloat16

    xT_d = nc.dram_tensor("xT", [BS, IN_DIM, L], f32r, kind="ExternalInput")
    wq_d = nc.dram_tensor("wq", [IN_DIM, D_MODEL], f32r, kind="ExternalInput")
    wk_d = nc.dram_tensor("wk", [IN_DIM, D_MODEL], f32r, kind="ExternalInput")
    wv_d = nc.dram_tensor("wv", [IN_DIM, D_MODEL], f32r, kind="ExternalInput")
    wf_d = nc.dram_tensor("wf", [4, IN_DIM, OUT_DIM], f32r, kind="ExternalInput")
    bq_d = nc.dram_tensor("bq", [IN_DIM, 4], f32, kind="ExternalInput")
    bk_d = nc.dram_tensor("bk", [IN_DIM, 4], f32, kind="ExternalInput")
    bv_d = nc.dram_tensor("bv", [1, D_MODEL], f32r, kind="ExternalInput")
    bf_d = nc.dram_tensor("bf", [1, OUT_DIM], f32r, kind="ExternalInput")
    mask_d = nc.dram_tensor("mask01", [2, 88, L], bf16, kind="ExternalInput")
    ones_d = nc.dram_tensor("ones", [1, IN_DIM], f32r, kind="ExternalInput")
    y_d = nc.dram_tensor("y", [BS, L, OUT_DIM], f32, kind="ExternalOutput")

    Copy = mybir.ActivationFunctionType.Copy
    Ident = mybir.ActivationFunctionType.Identity
    Exp = mybir.ActivationFunctionType.Exp
    Relu = mybir.ActivationFunctionType.Relu

    with tile.TileContext(nc) as tc:
        with (
            tc.tile_pool(name="consts", bufs=1) as cp,
            tc.tile_pool(name="xp", bufs=2) as xp,
            tc.tile_pool(name="qk", bufs=2) as qkp,
            tc.tile_pool(name="vp", bufs=2) as vp,
            tc.tile_pool(name="ptp", bufs=3) as ptp,
            tc.tile_pool(name="osb", bufs=2) as osbp,
            tc.tile_pool(name="recp", bufs=2) as recp,
            tc.tile_pool(name="yp", bufs=2) as yp,
            tc.tile_pool(name="ps_qo", bufs=2, space="PSUM") as pp_qo,
            tc.tile_pool(name="ps_vy", bufs=2, space="PSUM") as pp_vy,
            tc.tile_pool(name="ps_s", bufs=1, space="PSUM") as pp_s,
            tc.tile_pool(name="ps_b", bufs=2, space="PSUM") as pp_b,
        ):
            wq = cp.tile([IN_DIM, D_MODEL], f32r)
            nc.sync.dma_start(wq[:], wq_d.ap()[:])
            wk = cp.tile([IN_DIM, D_MODEL], f32r)
            nc.sync.dma_start(wk[:], wk_d.ap()[:])
            wv = cp.tile([IN_DIM, D_MODEL], f32r)
            nc.sync.dma_start(wv[:], wv_d.ap()[:])
            wf = cp.tile([IN_DIM, 4 * OUT_DIM], f32r)
            for c in range(4):
                nc.sync.dma_start(wf[:, 512 * c:512 * (c + 1)], wf_d.ap()[c])
            bq = cp.tile([IN_DIM, 4], f32)
            nc.sync.dma_start(bq[:], bq_d.ap()[:])
            bk = cp.tile([IN_DIM, 4], f32)
            nc.sync.dma_start(bk[:], bk_d.ap()[:])
            bv = cp.tile([1, D_MODEL], f32r)
            nc.sync.dma_start(bv[:], bv_d.ap()[:])
            bf_t = cp.tile([1, OUT_DIM], f32r)
            nc.sync.dma_start(bf_t[:], bf_d.ap()[:])
            mask01 = cp.tile([88, 2 * L], bf16)
            for kc in range(2):
                nc.sync.dma_start(mask01[:, L * kc:L * (kc + 1)], mask_d.ap()[kc])
            ones = cp.tile([1, IN_DIM], f32r)
            nc.sync.dma_start(ones[:], ones_d.ap()[:])

            for s in range(BS * repeat):
                s = s % BS
                xt = xp.tile([IN_DIM, L], f32r)
                nc.sync.dma_start(xt[:], xT_d.ap()[s])

                # q^T / k^T projections: psum [128, 176] per 128-chunk of
                # d_model; bias added during PSUM->SBUF copy on ScalarE.
                qt = qkp.tile([IN_DIM, 4 * L], f32r, name="qt")
                kt = qkp.tile([IN_DIM, 4 * L], f32r, name="kt")
                for w_t, b_t, dst in ((wq, bq, qt), (wk, bk, kt)):
                    for c in range(4):
                        pq = pp_qo.tile([IN_DIM, L], f32, name="pq", tag="qo")
                        nc.tensor.matmul(
                            pq[:], w_t[:, 128 * c:128 * (c + 1)], xt[:],
                            start=True, stop=True,
                        )
                        nc.scalar.activation(
                            dst[:, L * c:L * (c + 1)], pq[:], Ident,
                            bias=b_t[:, c:c + 1],
                        )

                # v: natural layout, keys on partitions, ones-augmented bf16
                va = []
                for rc in range(2):
                    pv = pp_vy.tile([88, D_MODEL], f32, name="pv", tag="vy")
                    nc.tensor.matmul(
                        pv[:], xt[:, 88 * rc:88 * (rc + 1)], wv[:],
                        start=True, stop=False,
                    )
                    nc.tensor.matmul(
                        pv[:], ones[:, 0:88], bv[:], start=False, stop=True,
                    )
                    vt = vp.tile([88, 8 * 65], bf16, name=f"va{rc}")
                    vv = vt[:].rearrange("p (h w) -> p h w", w=65)
                    pvv = pv[:].rearrange("p (h w) -> p h w", w=64)
                    nc.scalar.activation(vv[:, :, 0:64], pvv[:], Relu)
                    nc.gpsimd.memset(vv[:, :, 64:65], 1.0)
                    va.append(vt)

                osb = osbp.tile([IN_DIM, 4 * L], f32r, name="osb")
                for hp in range(4):
                    # S^T matmuls for the head pair interleaved: even head
                    # occupies PE rows 0-63, odd head rows 64-127 -> the
                    # weight loads/matmuls of the two heads overlap in the
                    # array (disjoint row groups).
                    sps = []
                    for kc in range(2):
                        for hs in range(2):
                            hr = 64 * hs
                            if kc == 0 and len(sps) < 2:
                                sps.append(pp_s.tile([88, 2 * L], f32,
                                                     name=f"sp{hs}"))
                            nc.tensor.matmul(
                                sps[hs][:, L * kc:L * (kc + 1)],
                                kt[hr:hr + 64,
                                   L * hp + 88 * kc:L * hp + 88 * (kc + 1)],
                                qt[hr:hr + 64, L * hp:L * (hp + 1)],
                                start=True, stop=True,
                            )
                    for hs in range(2):
                        h, hr = 2 * hp + hs, 64 * hs
                        pt = ptp.tile([88, 2 * L], bf16, name=f"pt{hs}")
                        nc.scalar.activation(pt[:], sps[hs][:], Exp)
                        nc.gpsimd.tensor_mul(pt[:], pt[:], mask01[:])

                        po = pp_qo.tile([65, L], f32, name="po", tag="qo")
                        for kc in range(2):
                            nc.tensor.matmul(
                                po[:], va[kc][:, 65 * h:65 * h + 65],
                                pt[:, L * kc:L * (kc + 1)],
                                start=(kc == 0), stop=(kc == 1),
                            )
                        rec = recp.tile([1, L], f32r, name="rec")
                        with nc.allow_low_precision(reason="f32r recip"):
                            nc.vector.reciprocal(rec[:], po[64:65, :])
                        pb = pp_b.tile([64, L], f32, name="pb")
                        nc.tensor.matmul(pb[:], ones[:, 0:64], rec[:],
                                         start=True, stop=True)
                        dst = osb[hr:hr + 64, L * hp:L * (hp + 1)]
                        if hs == 0:
                            nc.scalar.activation(dst, po[0:64, :], Copy)
                        else:
                            nc.vector.tensor_copy(dst, po[0:64, :])
                        nc.vector.tensor_mul(dst, dst, pb[:])

                for rc in range(2):
                    py = pp_vy.tile([88, OUT_DIM], f32, name="py", tag="vy")
                    for c in range(4):
                        nc.tensor.matmul(
                            py[:],
                            osb[:, L * c + 88 * rc:L * c + 88 * (rc + 1)],
                            wf[:, 512 * c:512 * (c + 1)],
                            start=(c == 0), stop=False,
                        )
                    nc.tensor.matmul(py[:], ones[:, 0:88], bf_t[:],
                                     start=False, stop=True)
                    ysb = yp.tile([88, OUT_DIM], f32, name="ysb")
                    if rc == 0:
                        nc.vector.tensor_copy(ysb[:], py[:])
                    else:
                        nc.scalar.activation(ysb[:], py[:], Copy)
                    nc.sync.dma_start(
                        y_d.ap()[s, 88 * rc:88 * (rc + 1), :], ysb[:],
                    )
    return nc


def _make_consts():
    frame = np.arange(L) // FRAME
    same_frame = frame[:, None] == frame[None, :]
    mask01 = np.where(same_frame & ~np.eye(L, dtype=bool), np.float32(0.0),
                      np.float32(1.0))
    import ml_dtypes
    return {
        "mask01": np.stack([mask01[0:88], mask01[88:176]]).astype(
            ml_dtypes.bfloat16),
        "ones": np.ones((1, IN_DIM), dtype=np.float32),
    }


_NC_CACHE = None


def kernel(x, Wq, bq, Wk, bk, Wv, bv, Wf, bf):
    global _NC_CACHE
    x = np.asarray(x, dtype=np.float32)
    if _NC_CACHE is None:
        _NC_CACHE = _build_nc()
    nc = _NC_CACHE

    consts = _make_consts()
    xT = np.ascontiguousarray(x.transpose(0, 2, 1))  # [B, 128, 176]
    base = {
        "wq": np.asarray(Wq, np.float32) * SCALE,  # fold 1/sqrt(H_DIM) into q
        "wk": np.asarray(Wk, np.float32),
        "wv": np.asarray(Wv, np.float32),
        "wf": np.ascontiguousarray(
            np.asarray(Wf, np.float32).reshape(4, IN_DIM, OUT_DIM)),
        "bq": np.ascontiguousarray(
            (np.asarray(bq, np.float32) * SCALE).reshape(4, IN_DIM).T),
        "bk": np.ascontiguousarray(np.asarray(bk, np.float32).reshape(4, IN_DIM).T),
        "bv": np.asarray(bv, np.float32).reshape(1, D_MODEL),
        "bf": np.asarray(bf, np.float32).reshape(1, OUT_DIM),
        **consts,
    }
    in_maps = [
        {**base, "xT": np.ascontiguousarray(xT[BS * c:BS * (c + 1)])}
        for c in range(N_CORES)
    ]
    res = run_bass_kernel_spmd(nc, in_maps, core_ids=list(range(N_CORES)))
    return np.concatenate([r["y"] for r in res.results], axis=0)


# revision 46
# speedup vs baseline: 397.9537x; 324.5230x over previous
"""Trainium2 Bass kernel for nn_MultiHeadedAttention_64665027608991.

Sparse (per-frame-masked) multi-head attention over B=512 samples, L=176
(8 frames x 22 joints), 8 heads x 64 dims, fp32 I/O.

Strategy: pure data parallel over batch (64 samples per NeuronCore x 8).
Per sample, fully unrolled:
  - x^T (host-pre-transposed) -> q^T/k^T via fp32r matmuls (tf32-class),
    biases folded into the PSUM->SBUF copy on ScalarE.
  - v natural layout with bias via K=1 ones matmul, ReLU on ScalarE into a
    ones-augmented bf16 tile (65 cols per head; col 64 = 1.0 for row sums).
  - scores S^T[k,q] per head via fp32r matmuls; the temporal mask is added
    in PSUM by a constant-matrix matmul (lhsT=I88, rhs=-100 on masked pairs);
    exp on ScalarE (no max subtraction: |scores| <= ~3) -> bf16 P^T.
  - O^T = [v|1]^T @ P^T (bf16): row 64 gives softmax denominators; recip on
    VectorE, broadcast via K=1 matmul, normalize on VectorE.
  - final projection from O^T slices (fp32r) + bias, DMA out.
"""

import sys

sys.path.insert(0, "/opt/trn_rl_repo")

import json

import numpy as np

import concourse.bass as bass
import concourse.tile as tile
from concourse import mybir
from concourse.bass_utils import run_bass_kernel_spmd

DT = mybir.dt

N_CORES = 8
B = 512
BS = B // N_CORES  # 64 samples per core
L = 176
FRAME = 22
NFRAME = 8
IN_DIM = 128
D_MODEL = 512
H_NUM = 8
H_DIM = 64
OUT_DIM = 512
SCALE = 1.0 / np.sqrt(np.float32(H_DIM))
NEG = -100.0  # exp(s + NEG) < 1e-40 for |s| < 10 -> masked weights vanish


# ---------------------------------------------------------------------------
# Workaround: the walrus build in this container rejects instructions with
# more than one sync-wait. Split extras onto single-wait EventSemaphore
# carriers on the same engine.
def _split_multiwaits(bir_json_bytes: bytes) -> bytes:
    j = json.loads(bir_json_bytes)
    n = [0]

    def fix_block(b):
        insts = b.get("instructions")
        if insts:
            out = []
            for inst in insts:
                si = inst.get("sync_info")
                waits = (si or {}).get("on_wait") or []
                if len(waits) > 1:
                    for w in waits[:-1]:
                        n[0] += 1
                        out.append({
                            "name": f"waitfix_{n[0]}",
                            "opcode": "EventSemaphore",
                            "engine": inst.get("engine"),
                            "ins": [],
                            "outs": [],
                            "sync_info": {"on_update": [], "on_wait": [w]},
                        })
                    si["on_wait"] = [waits[-1]]
                out.append(inst)
            b["instructions"] = out
        for sub in b.get("blocks", []) or []:
            fix_block(sub)

    for fn in j["functions"]:
        for blk in fn["blocks"]:
            fix_block(blk)
    return json.dumps(j).encode()


def _install_waitfix(nc):
    orig = nc.to_json_bytes
    nc.to_json_bytes = lambda: _split_multiwaits(orig())


CFG = {
    "xp": 2, "qk": 2, "vp": 2, "ptp": 3, "osb": 2, "recp": 2, "yp": 2,
    "ps_qo": 2, "ps_vy": 2, "ps_s": 1, "ps_b": 2,
}
MASK_MODE = "dve"  # "gpsimd" | "dve" | "pe"


def _build_nc(repeat=1):
    nc = bass.Bass(trn_type="TRN2", debug=False)
    _install_waitfix(nc)
    f32, f32r, bf16 = DT.float32, DT.float32r, DT.bfloat16

    xT_d = nc.dram_tensor("xT", [BS, IN_DIM, L], f32r, kind="ExternalInput")
    wq_d = nc.dram_tensor("wq", [IN_DIM, D_MODEL], f32r, kind="ExternalInput")
    wk_d = nc.dram_tensor("wk", [IN_DIM, D_MODEL], f32r, kind="ExternalInput")
    wv_d = nc.dram_tensor("wv", [IN_DIM, D_MODEL], f32r, kind="ExternalInput")
    wf_d = nc.dram_tensor("wf", [4, IN_DIM, OUT_DIM], f32r, kind="ExternalInput")
    bq_d = nc.dram_tensor("bq", [IN_DIM, 4], f32, kind="ExternalInput")
    bk_d = nc.dram_tensor("bk", [IN_DIM, 4], f32, kind="ExternalInput")
    bv_d = nc.dram_tensor("bv", [1, D_MODEL], f32r, kind="ExternalInput")
    bf_d = nc.dram_tensor("bf", [1, OUT_DIM], f32r, kind="ExternalInput")
    mask_d = nc.dram_tensor("mask01", [2, 88, L], bf16, kind="ExternalInput")
    madd_d = nc.dram_tensor("madd", [2, 88, L], bf16, kind="ExternalInput")
    i88_d = nc.dram_tensor("i88", [88, 88], bf16, kind="ExternalInput")
    ones_d = nc.dram_tensor("ones", [1, IN_DIM], f32r, kind="ExternalInput")
    y_d = nc.dram_tensor("y", [BS, L, OUT_DIM], f32, kind="ExternalOutput")

    Copy = mybir.ActivationFunctionType.Copy
    Ident = mybir.ActivationFunctionType.Identity
    Exp = mybir.ActivationFunctionType.Exp
    Relu = mybir.ActivationFunctionType.Relu

    with tile.TileContext(nc) as tc:
        with (
            tc.tile_pool(name="consts", bufs=1) as cp,
            tc.tile_pool(name="xp", bufs=CFG["xp"]) as xp,
            tc.tile_pool(name="qk", bufs=CFG["qk"]) as qkp,
            tc.tile_pool(name="vp", bufs=CFG["vp"]) as vp,
            tc.tile_pool(name="ptp", bufs=CFG["ptp"]) as ptp,
            tc.tile_pool(name="osb", bufs=CFG["osb"]) as osbp,
            tc.tile_pool(name="recp", bufs=CFG["recp"]) as recp,
            tc.tile_pool(name="yp", bufs=CFG["yp"]) as yp,
            tc.tile_pool(name="ps_qo", bufs=CFG["ps_qo"], space="PSUM") as pp_qo,
            tc.tile_pool(name="ps_vy", bufs=CFG["ps_vy"], space="PSUM") as pp_vy,
            tc.tile_pool(name="ps_s", bufs=CFG["ps_s"], space="PSUM") as pp_s,
            tc.tile_pool(name="ps_b", bufs=CFG["ps_b"], space="PSUM") as pp_b,
        ):
            wq = cp.tile([IN_DIM, D_MODEL], f32r)
            nc.sync.dma_start(wq[:], wq_d.ap()[:])
            wk = cp.tile([IN_DIM, D_MODEL], f32r)
            nc.sync.dma_start(wk[:], wk_d.ap()[:])
            wv = cp.tile([IN_DIM, D_MODEL], f32r)
            nc.sync.dma_start(wv[:], wv_d.ap()[:])
            wf = cp.tile([IN_DIM, 4 * OUT_DIM], f32r)
            for c in range(4):
                nc.sync.dma_start(wf[:, 512 * c:512 * (c + 1)], wf_d.ap()[c])
            bq = cp.tile([IN_DIM, 4], f32)
            nc.sync.dma_start(bq[:], bq_d.ap()[:])
            bk = cp.tile([IN_DIM, 4], f32)
            nc.sync.dma_start(bk[:], bk_d.ap()[:])
            bv = cp.tile([1, D_MODEL], f32r)
            nc.sync.dma_start(bv[:], bv_d.ap()[:])
            bf_t = cp.tile([1, OUT_DIM], f32r)
            nc.sync.dma_start(bf_t[:], bf_d.ap()[:])
            if MASK_MODE == "pe":
                madd = cp.tile([88, 2 * L], bf16)
                for kc in range(2):
                    nc.sync.dma_start(madd[:, L * kc:L * (kc + 1)],
                                      madd_d.ap()[kc])
                i88 = cp.tile([88, 88], bf16)
                nc.sync.dma_start(i88[:], i88_d.ap()[:])
            else:
                mask01 = cp.tile([88, 2 * L], bf16)
                for kc in range(2):
                    nc.sync.dma_start(mask01[:, L * kc:L * (kc + 1)],
                                      mask_d.ap()[kc])
            ones = cp.tile([1, IN_DIM], f32r)
            nc.sync.dma_start(ones[:], ones_d.ap()[:])

            for s in range(BS * repeat):
                s = s % BS
                xt = xp.tile([IN_DIM, L], f32r)
                nc.sync.dma_start(xt[:], xT_d.ap()[s])

                # q^T / k^T projections: psum [128, 176] per 128-chunk of
                # d_model; bias added during PSUM->SBUF copy on ScalarE.
                qt = qkp.tile([IN_DIM, 4 * L], f32r, name="qt")
                kt = qkp.tile([IN_DIM, 4 * L], f32r, name="kt")
                for w_t, b_t, dst in ((wq, bq, qt), (wk, bk, kt)):
                    for c in range(4):
                        pq = pp_qo.tile([IN_DIM, L], f32, name="pq", tag="qo")
                        nc.tensor.matmul(
                            pq[:], w_t[:, 128 * c:128 * (c + 1)], xt[:],
                            start=True, stop=True,
                        )
                        nc.scalar.activation(
                            dst[:, L * c:L * (c + 1)], pq[:], Ident,
                            bias=b_t[:, c:c + 1],
                        )

                # v: natural layout, keys on partitions, ones-augmented bf16
                va = []
                for rc in range(2):
                    pv = pp_vy.tile([88, D_MODEL], f32, name="pv", tag="vy")
                    nc.tensor.matmul(
                        pv[:], xt[:, 88 * rc:88 * (rc + 1)], wv[:],
                        start=True, stop=False,
                    )
                    nc.tensor.matmul(
                        pv[:], ones[:, 0:88], bv[:], start=False, stop=True,
                    )
                    vt = vp.tile([88, 8 * 65], bf16, name=f"va{rc}")
                    vv = vt[:].rearrange("p (h w) -> p h w", w=65)
                    pvv = pv[:].rearrange("p (h w) -> p h w", w=64)
                    nc.scalar.activation(vv[:, :, 0:64], pvv[:], Relu)
                    nc.gpsimd.memset(vv[:, :, 64:65], 1.0)
                    va.append(vt)

                osb = osbp.tile([IN_DIM, 4 * L], f32r, name="osb")
                for hp in range(4):
                    # S^T matmuls for the head pair interleaved: even head
                    # occupies PE rows 0-63, odd head rows 64-127 -> the
                    # weight loads/matmuls of the two heads overlap in the
                    # array (disjoint row groups).
                    sps = []
                    for kc in range(2):
                        for hs in range(2):
                            hr = 64 * hs
                            if kc == 0 and len(sps) < 2:
                                sps.append(pp_s.tile([88, 2 * L], f32,
                                                     name=f"sp{hs}"))
                            nc.tensor.matmul(
                                sps[hs][:, L * kc:L * (kc + 1)],
                                kt[hr:hr + 64,
                                   L * hp + 88 * kc:L * hp + 88 * (kc + 1)],
                                qt[hr:hr + 64, L * hp:L * (hp + 1)],
                                start=True, stop=(MASK_MODE != "pe"),
                            )
                            if MASK_MODE == "pe":
                                nc.tensor.matmul(
                                    sps[hs][:, L * kc:L * (kc + 1)],
                                    i88[:], madd[:, L * kc:L * (kc + 1)],
                                    start=False, stop=True,
                                )
                    for hs in range(2):
                        h, hr = 2 * hp + hs, 64 * hs
                        pt = ptp.tile([88, 2 * L], bf16, name=f"pt{hs}")
                        nc.scalar.activation(pt[:], sps[hs][:], Exp)
                        if MASK_MODE == "gpsimd":
                            nc.gpsimd.tensor_mul(pt[:], pt[:], mask01[:])
                        elif MASK_MODE == "dve":
                            nc.vector.tensor_mul(pt[:], pt[:], mask01[:])

                        po = pp_qo.tile([65, L], f32, name="po", tag="qo")
                        for kc in range(2):
                            nc.tensor.matmul(
                                po[:], va[kc][:, 65 * h:65 * h + 65],
                                pt[:, L * kc:L * (kc + 1)],
                                start=(kc == 0), stop=(kc == 1),
                            )
                        rec = recp.tile([1, L], f32r, name="rec")
                        with nc.allow_low_precision(reason="f32r recip"):
                            nc.vector.reciprocal(rec[:], po[64:65, :])
                        pb = pp_b.tile([64, L], f32, name="pb")
                        nc.tensor.matmul(pb[:], ones[:, 0:64], rec[:],
                                         start=True, stop=True)
                        dst = osb[hr:hr + 64, L * hp:L * (hp + 1)]
                        if hs == 0:
                            nc.scalar.activation(dst, po[0:64, :], Copy)
                        else:
                            nc.vector.tensor_copy(dst, po[0:64, :])
                        nc.vector.tensor_mul(dst, dst, pb[:])

                for rc in range(2):
                    py = pp_vy.tile([88, OUT_DIM], f32, name="py", tag="vy")
                    for c in range(4):
                        nc.tensor.matmul(
                            py[:],
                            osb[:, L * c + 88 * rc:L * c + 88 * (rc + 1)],
                            wf[:, 512 * c:512 * (c + 1)],
                            start=(c == 0), stop=False,
                        )
                    nc.tensor.matmul(py[:], ones[:, 0:88], bf_t[:],
                                     start=False, stop=True)
                    ysb = yp.tile([88, OUT_DIM], f32, name="ysb")
                    if rc == 0:
                        nc.vector.tensor_copy(ysb[:], py[:])
                    else:
                        nc.scalar.activation(ysb[:], py[:], Copy)
                    nc.sync.dma_start(
                        y_d.ap()[s, 88 * rc:88 * (rc + 1), :], ysb[:],
                    )
    return nc


def _make_consts():
    frame = np.arange(L) // FRAME
    same_frame = frame[:, None] == frame[None, :]
    mask01 = np.where(same_frame & ~np.eye(L, dtype=bool), np.float32(0.0),
                      np.float32(1.0))
    madd = np.where(same_frame & ~np.eye(L, dtype=bool), np.float32(NEG),
                    np.float32(0.0))
    import ml_dtypes
    return {
        "mask01": np.stack([mask01[0:88], mask01[88:176]]).astype(
            ml_dtypes.bfloat16),
        "madd": np.stack([madd[0:88], madd[88:176]]).astype(ml_dtypes.bfloat16),
        "i88": np.eye(88, dtype=np.float32).astype(ml_dtypes.bfloat16),
        "ones": np.ones((1, IN_DIM), dtype=np.float32),
    }


_NC_CACHE = None


def kernel(x, Wq, bq, Wk, bk, Wv, bv, Wf, bf):
    global _NC_CACHE
    x = np.asarray(x, dtype=np.float32)
    if _NC_CACHE is None:
        _NC_CACHE = _build_nc()
    nc = _NC_CACHE

    consts = _make_consts()
    xT = np.ascontiguousarray(x.transpose(0, 2, 1))  # [B, 128, 176]
    base = {
        "wq": np.asarray(Wq, np.float32) * SCALE,  # fold 1/sqrt(H_DIM) into q
        "wk": np.asarray(Wk, np.float32),
        "wv": np.asarray(Wv, np.float32),
        "wf": np.ascontiguousarray(
            np.asarray(Wf, np.float32).reshape(4, IN_DIM, OUT_DIM)),
        "bq": np.ascontiguousarray(
            (np.asarray(bq, np.float32) * SCALE).reshape(4, IN_DIM).T),
        "bk": np.ascontiguousarray(np.asarray(bk, np.float32).reshape(4, IN_DIM).T),
        "bv": np.asarray(bv, np.float32).reshape(1, D_MODEL),
        "bf": np.asarray(bf, np.float32).reshape(1, OUT_DIM),
        **consts,
    }
    in_maps = [
        {**base, "xT": np.ascontiguousarray(xT[BS * c:BS * (c + 1)])}
        for c in range(N_CORES)
    ]
    global _last_in_maps
    _last_in_maps = in_maps
    res = run_bass_kernel_spmd(nc, in_maps, core_ids=list(range(N_CORES)))
    return np.concatenate([r["y"] for r in res.results], axis=0)


_last_in_maps = None
